# revision 3
# baseline (speedup 1.0000x reference)
"""Depthwise 7x7 conv (stride 1, pad 3) on 8 NeuronCores via Bass.

Strategy: channel-sharded SPMD (48 channels/core).  Per channel, conv along H
is a banded matmul on TensorE (stationary = banded filter matrix G, moving =
X rows); the 7 kw taps accumulate in PSUM via free-dim-shifted rhs slices.

v2 changes vs v1:
- The per-channel 24-row runt is computed for 4 channels at once in a SINGLE
  plain matmul per tap (block-diagonal G_runt, K=128/M=96) instead of 4
  tile_position-packed matmuls that share PE streaming bandwidth.
  1680 -> 1428 matmuls per core.
- Outputs stored as fp16 (half the write traffic); host converts to f32.
- X is pre-arranged on host into window-major layout [c, 128, 5, 512] so the
  big per-channel load is 4 KB contiguous per partition; Y uses a matching
  [c, 128, 4*512] layout (un-permuted on host).
- Runt windows live in a dedicated zeroed tile, shortening x-buffer lifetimes.
"""

import numpy as np

import concourse.bacc as bacc
import concourse.mybir as mybir
import concourse.tile as tile
from concourse.ap import AP
from concourse.bass_utils import run_bass_kernel_spmd

C, H, W_DIM = 384, 512, 512
KH = KW = 7
PAD = 3
N_CORES = 8
CPC = C // N_CORES  # 48 channels per core
NGRP = CPC // 4     # 12 runt groups per core

GW = 125   # master banded-matrix width (full blocks)
GR = 96    # runt band width: 4 channels x 24 output rows
HP = 520   # padded rows per channel (3 zero top + 512 + 5 zero tail)
MT = 122   # output rows per full tile
NFULL = 4  # full tiles per channel
MR = H - NFULL * MT  # runt output rows per channel (24)
KR = MR + PAD        # runt contraction rows (27)
NWIN = NFULL + 1     # windows in the host X layout (4 full + 1 runt slot)

import os as _os
N_XBUF = int(_os.environ.get("N_XBUF", "8"))
N_OBUF = int(_os.environ.get("N_OBUF", "6"))

F32 = mybir.dt.float32
F16 = mybir.dt.float16
NP_IN = np.float16


def _ap(base, dims, extra_off=0):
    return AP(tensor=base.tensor, offset=base.offset + extra_off, ap=list(dims))


def emit_body(nc, pools, x_ts, o_ts, xw_dram, g_dram, gr_dram, y_dram, yr_dram):
    g_pool, gr_pool, xr_pool, or_pool, ps_pool = pools
    w = W_DIM
    kws = [PAD] + [k for k in range(KW) if k != PAD]
    ti = 0
    oi = 0
    for grp in range(NGRP):
        xr_t = None
        gr_t = None
        for i in range(4):
            c = grp * 4 + i
            g_t = g_pool.tile([128, KW * GW], F16, tag="g", name="g_t")
            nc.sync.dma_start(g_t[:], g_dram[c])
            x_t = x_ts[ti % len(x_ts)]
            ti += 1
            # 4 overlapping 128-row windows, 4KB contiguous per partition
            nc.sync.dma_start(
                x_t[:], _ap(xw_dram[c], [[NWIN * w, 128], [1, NFULL * w]])
            )
            if i == 0:
                # group-shared runt resources, issued early enough to be
                # loaded before the runt matmuls ~30 matmuls later
                gr_t = gr_pool.tile([128, KW * GR], F16, tag="gr", name="gr_t")
                nc.sync.dma_start(gr_t[:], gr_dram[grp])
                xr_t = xr_pool.tile([128, w], F16, tag="xr", name="xr_t")
                for j in range(4):
                    cj = grp * 4 + j
                    nc.sync.dma_start(
                        xr_t[32 * j : 32 * j + KR, :],
                        _ap(xw_dram[cj], [[NWIN * w, KR], [1, w]],
                            extra_off=NFULL * w),
                    )
            o_t = o_ts[oi % len(o_ts)]
            oi += 1
            for t in range(NFULL):
                ps_t = ps_pool.tile([128, w], F32, tag="ps", name="ps_t")
                # kw=PAD (shift 0) first: full-width start=True sets
                # has_written for the bank; shifted kws accumulate subranges.
                for idx, kw in enumerate(kws):
                    s = kw - PAD
                    w_lo = max(0, -s)
                    w_hi = w + min(0, -s)
                    lhs = g_t[:128, kw * GW + PAD : kw * GW + PAD + MT]
                    rhs = x_t[:128, t * w + w_lo + s : t * w + w_hi + s]
                    nc.tensor.matmul(
                        ps_t[:MT, w_lo:w_hi], lhs, rhs,
                        start=(idx == 0), stop=(idx == KW - 1),
                    )
                nc.vector.tensor_copy(o_t[:MT, t * w : (t + 1) * w], ps_t[:MT, :])
            nc.scalar.dma_start(
                _ap(y_dram[c], [[NFULL * w, MT], [1, NFULL * w]]), o_t[:MT, :]
            )
        # one plain matmul per tap covers all 4 runts (block-diagonal G_runt)
        ps_r = ps_pool.tile([128, w], F32, tag="ps", name="ps_r")
        for idx, kw in enumerate(kws):
            s = kw - PAD
            w_lo = max(0, -s)
            w_hi = w + min(0, -s)
            lhs = gr_t[:128, kw * GR : kw * GR + GR]
            rhs = xr_t[:128, w_lo + s : w_hi + s]
            nc.tensor.matmul(
                ps_r[:GR, w_lo:w_hi], lhs, rhs,
                start=(idx == 0), stop=(idx == KW - 1),
            )
        o_r = or_pool.tile([128, w], F16, tag="or", name="o_r")
        nc.vector.tensor_copy(o_r[:GR, :], ps_r[:GR, :])
        nc.scalar.dma_start(yr_dram[grp], o_r[:GR, :])


def build_nc(cpc=CPC):
    w = W_DIM
    nc = bacc.Bacc(None, target_bir_lowering=False)

    xw_dram = nc.dram_tensor("XW", [cpc, 128, NWIN * w], F16, kind="ExternalInput")
    g_dram = nc.dram_tensor("G", [cpc, 128, KW * GW], F16, kind="ExternalInput")
    gr_dram = nc.dram_tensor("GRT", [NGRP, 128, KW * GR], F16, kind="ExternalInput")
    y_dram = nc.dram_tensor("Y", [cpc, 128, NFULL * w], F16, kind="ExternalOutput")
    yr_dram = nc.dram_tensor("YR", [NGRP, GR, w], F16, kind="ExternalOutput")

    with tile.TileContext(nc) as tc:
        with (
            tc.tile_pool(name="xw", bufs=1) as x_pool,
            tc.tile_pool(name="g", bufs=6) as g_pool,
            tc.tile_pool(name="gr", bufs=3) as gr_pool,
            tc.tile_pool(name="xr", bufs=3) as xr_pool,
            tc.tile_pool(name="ob", bufs=1) as o_pool,
            tc.tile_pool(name="orp", bufs=3) as or_pool,
            tc.tile_pool(name="ps", bufs=7, space="PSUM") as ps_pool,
        ):
            x_ts = [
                x_pool.tile([128, NFULL * w], F16, tag=f"x{i}", name=f"x{i}")
                for i in range(N_XBUF)
            ]
            o_ts = [
                o_pool.tile([128, NFULL * w], F16, tag=f"o{i}", name=f"o{i}")
                for i in range(N_OBUF)
            ]
            # runt tiles keep partitions 27..31 etc. unwritten; zero them so
            # zero-weight lanes multiply 0, not stale NaN bits
            xr_zs = [
                xr_pool.tile([128, w], F16, tag="xr", name=f"xrz{i}")
                for i in range(3)
            ]
            for t in xr_zs:
                nc.vector.memset(t[:, :], 0.0)
            pools = (g_pool, gr_pool, xr_pool, or_pool, ps_pool)
            emit_body(nc, pools, x_ts, o_ts,
                      xw_dram, g_dram, gr_dram, y_dram, yr_dram)

    nc.compile()
    return nc


def build_g(wf):
    """wf: (C, 7, 7) filters -> (C, 128, 7, GW) fp16 banded matrices.

    G[c, j, kw, m2] = wf[c, j - m2 + 3, kw] where valid (0..6), else 0.
    Full tiles slice at g_off=PAD: lhsT[j, m] = wf[j - m] over padded input.
    """
    c = wf.shape[0]
    g = np.zeros((c, 128, KW, GW), dtype=NP_IN)
    js = np.arange(128)
    for kh in range(KH):
        m2 = js + 3 - kh
        mask = (m2 >= 0) & (m2 < GW)
        g[:, js[mask], :, m2[mask]] = wf[None, :, kh, :].astype(NP_IN)
    return g


def build_gr(wf):
    """wf: (C, 7, 7) -> (NGRP_total, 128, 7, GR) block-diagonal runt bands.

    Block i of group grp: rows 32i+jp (jp<27), cols 24i+mp (mp<24), value
    wf[4*grp+i, jp-mp, kw] when 0 <= jp-mp <= 6.
    """
    c = wf.shape[0]
    ngrp = c // 4
    gr = np.zeros((ngrp, 128, KW, GR), dtype=NP_IN)
    for i in range(4):
        for kh in range(KH):
            mp = np.arange(MR)
            jp = mp + kh
            sel = jp < KR
            # advanced indices separated by a slice: result axes are
            # (pair, group, kw), so broadcast the (group, kw) value into it
            gr[:, 32 * i + jp[sel], :, 24 * i + mp[sel]] = (
                wf[i::4, kh, :].astype(NP_IN)
            )
    return gr


def build_xw(x):
    """(C, H, W) f32 -> (C, 128, NWIN, W) fp16 window-major layout.

    Window t<4: padded rows 122t..122t+127 (pad = 3 zero rows on top).
    Window 4: padded rows 488..514 in slots 0..26, zeros elsewhere.
    """
    c, h, w = x.shape
    xp = np.zeros((c, HP, w), dtype=NP_IN)
    xp[:, PAD : PAD + h] = x.astype(NP_IN)
    xw = np.zeros((c, 128, NWIN, w), dtype=NP_IN)
    for t in range(NFULL):
        xw[:, :, t, :] = xp[:, MT * t : MT * t + 128, :]
    xw[:, :KR, NFULL, :] = xp[:, NFULL * MT : NFULL * MT + KR, :]
    return xw


_NC_CACHE = {}


def _get_nc():
    if CPC not in _NC_CACHE:
        _NC_CACHE[CPC] = build_nc(CPC)
    return _NC_CACHE[CPC]


def run(X, W, **spmd_kwargs):
    X = np.asarray(X, dtype=np.float32)
    W = np.asarray(W, dtype=np.float32)
    wf = np.ascontiguousarray(W[:, 0])  # (C, 7, 7)
    g_all = build_g(wf).reshape(C, 128, KW * GW)
    gr_all = build_gr(wf).reshape(C // 4, 128, KW * GR)
    xw_all = build_xw(X).reshape(C, 128, NWIN * W_DIM)

    nc = _get_nc()
    in_maps = []
    for core in range(N_CORES):
        c0 = core * CPC
        g0 = core * NGRP
        in_maps.append(
            {
                "XW": np.ascontiguousarray(xw_all[c0 : c0 + CPC]),
                "G": np.ascontiguousarray(g_all[c0 : c0 + CPC]),
                "GRT": np.ascontiguousarray(gr_all[g0 : g0 + NGRP]),
            }
        )
    res = run_bass_kernel_spmd(nc, in_maps, core_ids=list(range(N_CORES)),
                               **spmd_kwargs)
    y = np.empty((C, H, W_DIM), dtype=np.float32)
    for core in range(N_CORES):
        c0 = core * CPC
        yw = res.results[core]["Y"].reshape(CPC, 128, NFULL, W_DIM)
        yr = res.results[core]["YR"].reshape(NGRP, 4, MR, W_DIM)
        for t in range(NFULL):
            y[c0 : c0 + CPC, MT * t : MT * t + MT] = yw[:, :MT, t]
        y[c0 : c0 + CPC, NFULL * MT :] = yr.reshape(CPC, MR, W_DIM)
    return y, res


def kernel(X, W):
    return run(X, W)[0]


# revision 4
# speedup vs baseline: 1.7987x; 1.7987x over previous
"""Depthwise 7x7 conv (stride 1, pad 3) on 8 NeuronCores via Bass.

Strategy: channel-sharded SPMD (48 channels/core).  Per channel, conv along H
is a banded matmul on TensorE (stationary = banded filter matrix G, moving =
X rows); the 7 kw taps accumulate in PSUM via free-dim-shifted rhs slices.

v2 changes vs v1:
- The per-channel 24-row runt is computed for 4 channels at once in a SINGLE
  plain matmul per tap (block-diagonal G_runt, K=128/M=96) instead of 4
  tile_position-packed matmuls that share PE streaming bandwidth.
  1680 -> 1428 matmuls per core.
- Outputs stored as fp16 (half the write traffic); host converts to f32.
- X is pre-arranged on host into window-major layout [c, 128, 5, 512] so the
  big per-channel load is 4 KB contiguous per partition; Y uses a matching
  [c, 128, 4*512] layout (un-permuted on host).
- Runt windows live in a dedicated zeroed tile, shortening x-buffer lifetimes.
"""

import numpy as np

import concourse.bacc as bacc
import concourse.mybir as mybir
import concourse.tile as tile
from concourse.ap import AP
from concourse.bass_utils import run_bass_kernel_spmd

C, H, W_DIM = 384, 512, 512
KH = KW = 7
PAD = 3
N_CORES = 8
CPC = C // N_CORES  # 48 channels per core
NGRP = CPC // 4     # 12 runt groups per core

GW = 125   # master banded-matrix width (full blocks)
GR = 96    # runt band width: 4 channels x 24 output rows
HP = 520   # padded rows per channel (3 zero top + 512 + 5 zero tail)
MT = 122   # output rows per full tile
NFULL = 4  # full tiles per channel
MR = H - NFULL * MT  # runt output rows per channel (24)
KR = MR + PAD        # runt contraction rows (27)
NWIN = NFULL + 1     # windows in the host X layout (4 full + 1 runt slot)

import os as _os
N_XBUF = int(_os.environ.get("N_XBUF", "8"))
N_OBUF = int(_os.environ.get("N_OBUF", "6"))

F32 = mybir.dt.float32
F16 = mybir.dt.float16
NP_IN = np.float16


def _ap(base, dims, extra_off=0):
    return AP(tensor=base.tensor, offset=base.offset + extra_off, ap=list(dims))


def emit_body(nc, pools, x_ts, o_ts, xw_dram, g_dram, gr_dram, y_dram, yr_dram):
    g_pool, gr_pool, xr_pool, or_pool, ps_pool = pools
    w = W_DIM
    kws = [PAD] + [k for k in range(KW) if k != PAD]
    ti = 0
    oi = 0
    for grp in range(NGRP):
        xr_t = None
        gr_t = None
        for i in range(4):
            c = grp * 4 + i
            g_t = g_pool.tile([128, KW * GW], F16, tag="g", name="g_t")
            nc.sync.dma_start(g_t[:], g_dram[c])
            x_t = x_ts[ti % len(x_ts)]
            ti += 1
            # 4 overlapping 128-row windows, 4KB contiguous per partition
            nc.sync.dma_start(
                x_t[:], _ap(xw_dram[c], [[NWIN * w, 128], [1, NFULL * w]])
            )
            if i == 0:
                # group-shared runt resources, issued early enough to be
                # loaded before the runt matmuls ~30 matmuls later
                gr_t = gr_pool.tile([128, KW * GR], F16, tag="gr", name="gr_t")
                nc.sync.dma_start(gr_t[:], gr_dram[grp])
                xr_t = xr_pool.tile([128, w], F16, tag="xr", name="xr_t")
                for j in range(4):
                    cj = grp * 4 + j
                    nc.sync.dma_start(
                        xr_t[32 * j : 32 * j + KR, :],
                        _ap(xw_dram[cj], [[NWIN * w, KR], [1, w]],
                            extra_off=NFULL * w),
                    )
            o_t = o_ts[oi % len(o_ts)]
            oi += 1
            for t in range(NFULL):
                ps_t = ps_pool.tile([128, w], F32, tag="ps", name="ps_t")
                # kw=PAD (shift 0) first: full-width start=True sets
                # has_written for the bank; shifted kws accumulate subranges.
                for idx, kw in enumerate(kws):
                    s = kw - PAD
                    w_lo = max(0, -s)
                    w_hi = w + min(0, -s)
                    lhs = g_t[:128, kw * GW + PAD : kw * GW + PAD + MT]
                    rhs = x_t[:128, t * w + w_lo + s : t * w + w_hi + s]
                    nc.tensor.matmul(
                        ps_t[:MT, w_lo:w_hi], lhs, rhs,
                        start=(idx == 0), stop=(idx == KW - 1),
                    )
                nc.vector.tensor_copy(o_t[:MT, t * w : (t + 1) * w], ps_t[:MT, :])
            # full-tile src + plain tensor-slice dst: HWDGE spreads this
            # across all 16 SDMA engines (a partial 122-row AP collapses
            # onto 2 engines). Rows 122..127 are junk the host ignores.
            nc.scalar.dma_start(y_dram[c], o_t[:, :])
        # one plain matmul per tap covers all 4 runts (block-diagonal G_runt)
        ps_r = ps_pool.tile([128, w], F32, tag="ps", name="ps_r")
        for idx, kw in enumerate(kws):
            s = kw - PAD
            w_lo = max(0, -s)
            w_hi = w + min(0, -s)
            lhs = gr_t[:128, kw * GR : kw * GR + GR]
            rhs = xr_t[:128, w_lo + s : w_hi + s]
            nc.tensor.matmul(
                ps_r[:GR, w_lo:w_hi], lhs, rhs,
                start=(idx == 0), stop=(idx == KW - 1),
            )
        o_r = or_pool.tile([128, w], F16, tag="or", name="o_r")
        nc.vector.tensor_copy(o_r[:GR, :], ps_r[:GR, :])
        nc.scalar.dma_start(yr_dram[grp], o_r[:GR, :])


def build_nc(cpc=CPC):
    w = W_DIM
    nc = bacc.Bacc(None, target_bir_lowering=False)

    xw_dram = nc.dram_tensor("XW", [cpc, 128, NWIN * w], F16, kind="ExternalInput")
    g_dram = nc.dram_tensor("G", [cpc, 128, KW * GW], F16, kind="ExternalInput")
    gr_dram = nc.dram_tensor("GRT", [NGRP, 128, KW * GR], F16, kind="ExternalInput")
    y_dram = nc.dram_tensor("Y", [cpc, 128, NFULL * w], F16, kind="ExternalOutput")
    yr_dram = nc.dram_tensor("YR", [NGRP, GR, w], F16, kind="ExternalOutput")

    with tile.TileContext(nc) as tc:
        with (
            tc.tile_pool(name="xw", bufs=1) as x_pool,
            tc.tile_pool(name="g", bufs=6) as g_pool,
            tc.tile_pool(name="gr", bufs=3) as gr_pool,
            tc.tile_pool(name="xr", bufs=3) as xr_pool,
            tc.tile_pool(name="ob", bufs=1) as o_pool,
            tc.tile_pool(name="orp", bufs=3) as or_pool,
            tc.tile_pool(name="ps", bufs=7, space="PSUM") as ps_pool,
        ):
            x_ts = [
                x_pool.tile([128, NFULL * w], F16, tag=f"x{i}", name=f"x{i}")
                for i in range(N_XBUF)
            ]
            o_ts = [
                o_pool.tile([128, NFULL * w], F16, tag=f"o{i}", name=f"o{i}")
                for i in range(N_OBUF)
            ]
            # runt tiles keep partitions 27..31 etc. unwritten; zero them so
            # zero-weight lanes multiply 0, not stale NaN bits
            xr_zs = [
                xr_pool.tile([128, w], F16, tag="xr", name=f"xrz{i}")
                for i in range(3)
            ]
            for t in xr_zs:
                nc.vector.memset(t[:, :], 0.0)
            pools = (g_pool, gr_pool, xr_pool, or_pool, ps_pool)
            emit_body(nc, pools, x_ts, o_ts,
                      xw_dram, g_dram, gr_dram, y_dram, yr_dram)

    nc.compile()
    return nc


def build_g(wf):
    """wf: (C, 7, 7) filters -> (C, 128, 7, GW) fp16 banded matrices.

    G[c, j, kw, m2] = wf[c, j - m2 + 3, kw] where valid (0..6), else 0.
    Full tiles slice at g_off=PAD: lhsT[j, m] = wf[j - m] over padded input.
    """
    c = wf.shape[0]
    g = np.zeros((c, 128, KW, GW), dtype=NP_IN)
    js = np.arange(128)
    for kh in range(KH):
        m2 = js + 3 - kh
        mask = (m2 >= 0) & (m2 < GW)
        g[:, js[mask], :, m2[mask]] = wf[None, :, kh, :].astype(NP_IN)
    return g


def build_gr(wf):
    """wf: (C, 7, 7) -> (NGRP_total, 128, 7, GR) block-diagonal runt bands.

    Block i of group grp: rows 32i+jp (jp<27), cols 24i+mp (mp<24), value
    wf[4*grp+i, jp-mp, kw] when 0 <= jp-mp <= 6.
    """
    c = wf.shape[0]
    ngrp = c // 4
    gr = np.zeros((ngrp, 128, KW, GR), dtype=NP_IN)
    for i in range(4):
        for kh in range(KH):
            mp = np.arange(MR)
            jp = mp + kh
            sel = jp < KR
            # advanced indices separated by a slice: result axes are
            # (pair, group, kw), so broadcast the (group, kw) value into it
            gr[:, 32 * i + jp[sel], :, 24 * i + mp[sel]] = (
                wf[i::4, kh, :].astype(NP_IN)
            )
    return gr


def build_xw(x):
    """(C, H, W) f32 -> (C, 128, NWIN, W) fp16 window-major layout.

    Window t<4: padded rows 122t..122t+127 (pad = 3 zero rows on top).
    Window 4: padded rows 488..514 in slots 0..26, zeros elsewhere.
    """
    c, h, w = x.shape
    xp = np.zeros((c, HP, w), dtype=NP_IN)
    xp[:, PAD : PAD + h] = x.astype(NP_IN)
    xw = np.zeros((c, 128, NWIN, w), dtype=NP_IN)
    for t in range(NFULL):
        xw[:, :, t, :] = xp[:, MT * t : MT * t + 128, :]
    xw[:, :KR, NFULL, :] = xp[:, NFULL * MT : NFULL * MT + KR, :]
    return xw


_NC_CACHE = {}


def _get_nc():
    if CPC not in _NC_CACHE:
        _NC_CACHE[CPC] = build_nc(CPC)
    return _NC_CACHE[CPC]


def run(X, W, **spmd_kwargs):
    X = np.asarray(X, dtype=np.float32)
    W = np.asarray(W, dtype=np.float32)
    wf = np.ascontiguousarray(W[:, 0])  # (C, 7, 7)
    g_all = build_g(wf).reshape(C, 128, KW * GW)
    gr_all = build_gr(wf).reshape(C // 4, 128, KW * GR)
    xw_all = build_xw(X).reshape(C, 128, NWIN * W_DIM)

    nc = _get_nc()
    in_maps = []
    for core in range(N_CORES):
        c0 = core * CPC
        g0 = core * NGRP
        in_maps.append(
            {
                "XW": np.ascontiguousarray(xw_all[c0 : c0 + CPC]),
                "G": np.ascontiguousarray(g_all[c0 : c0 + CPC]),
                "GRT": np.ascontiguousarray(gr_all[g0 : g0 + NGRP]),
            }
        )
    res = run_bass_kernel_spmd(nc, in_maps, core_ids=list(range(N_CORES)),
                               **spmd_kwargs)
    y = np.empty((C, H, W_DIM), dtype=np.float32)
    for core in range(N_CORES):
        c0 = core * CPC
        yw = res.results[core]["Y"].reshape(CPC, 128, NFULL, W_DIM)
        yr = res.results[core]["YR"].reshape(NGRP, 4, MR, W_DIM)
        for t in range(NFULL):
            y[c0 : c0 + CPC, MT * t : MT * t + MT] = yw[:, :MT, t]
        y[c0 : c0 + CPC, NFULL * MT :] = yr.reshape(CPC, MR, W_DIM)
    return y, res


def kernel(X, W):
    return run(X, W)[0]


# revision 7
# speedup vs baseline: 1.8064x; 1.0043x over previous
"""Depthwise 7x7 conv (stride 1, pad 3) on 8 NeuronCores via Bass.

Strategy: channel-sharded SPMD (48 channels/core).  Per channel, conv along H
is a banded matmul on TensorE (stationary = banded filter matrix G, moving =
X rows); the 7 kw taps accumulate in PSUM via free-dim-shifted rhs slices.

v2 changes vs v1:
- The per-channel 24-row runt is computed for 4 channels at once in a SINGLE
  plain matmul per tap (block-diagonal G_runt, K=128/M=96) instead of 4
  tile_position-packed matmuls that share PE streaming bandwidth.
  1680 -> 1428 matmuls per core.
- Outputs stored as fp16 (half the write traffic); host converts to f32.
- X is pre-arranged on host into window-major layout [c, 128, 5, 512] so the
  big per-channel load is 4 KB contiguous per partition; Y uses a matching
  [c, 128, 4*512] layout (un-permuted on host).
- Runt windows live in a dedicated zeroed tile, shortening x-buffer lifetimes.
"""

import numpy as np

import concourse.bacc as bacc
import concourse.mybir as mybir
import concourse.tile as tile
from concourse.ap import AP
from concourse.bass_utils import run_bass_kernel_spmd

C, H, W_DIM = 384, 512, 512
KH = KW = 7
PAD = 3
N_CORES = 8
CPC = C // N_CORES  # 48 channels per core
NGRP = CPC // 4     # 12 runt groups per core

GW = 125   # master banded-matrix width (full blocks)
GR = 96    # runt band width: 4 channels x 24 output rows
HP = 520   # padded rows per channel (3 zero top + 512 + 5 zero tail)
MT = 122   # output rows per full tile
NFULL = 4  # full tiles per channel
MR = H - NFULL * MT  # runt output rows per channel (24)
KR = MR + PAD        # runt contraction rows (27)
NWIN = NFULL + 1     # windows in the host X layout (4 full + 1 runt slot)

import os as _os
N_XBUF = int(_os.environ.get("N_XBUF", "8"))
N_OBUF = int(_os.environ.get("N_OBUF", "10"))

F32 = mybir.dt.float32
F16 = mybir.dt.float16
NP_IN = np.float16


def _ap(base, dims, extra_off=0):
    return AP(tensor=base.tensor, offset=base.offset + extra_off, ap=list(dims))


def emit_body(nc, pools, x_ts, o_ts, xw_dram, g_dram, gr_dram, y_dram, yr_dram):
    g_pool, gr_pool, xr_pool, or_pool, ps_pool = pools
    w = W_DIM
    kws = [PAD] + [k for k in range(KW) if k != PAD]
    ti = 0
    oi = 0
    for grp in range(NGRP):
        xr_t = None
        gr_t = None
        for i in range(4):
            c = grp * 4 + i
            g_t = g_pool.tile([128, KW * GW], F16, tag="g", name="g_t")
            x_t = x_ts[ti % len(x_ts)]
            ti += 1
            if grp == 0 and i == 0:
                # split the very first channel's loads so the first matmul
                # (tap kw=PAD on window 0) waits on ~160KB, not ~736KB
                k3 = PAD * GW
                nc.sync.dma_start(g_t[:, k3 : k3 + GW],
                                  _ap(g_dram[c], [[KW * GW, 128], [1, GW]],
                                      extra_off=k3))
                nc.sync.dma_start(
                    x_t[:, 0:w], _ap(xw_dram[c], [[NWIN * w, 128], [1, w]])
                )
                nc.sync.dma_start(g_t[:, 0:k3],
                                  _ap(g_dram[c], [[KW * GW, 128], [1, k3]]))
                nc.sync.dma_start(
                    g_t[:, k3 + GW :],
                    _ap(g_dram[c], [[KW * GW, 128], [1, (KW - PAD - 1) * GW]],
                        extra_off=k3 + GW),
                )
                nc.sync.dma_start(
                    x_t[:, w : NFULL * w],
                    _ap(xw_dram[c], [[NWIN * w, 128], [1, (NFULL - 1) * w]],
                        extra_off=w),
                )
            else:
                nc.sync.dma_start(g_t[:], g_dram[c])
                # 4 overlapping 128-row windows, 4KB contiguous per partition
                nc.sync.dma_start(
                    x_t[:], _ap(xw_dram[c], [[NWIN * w, 128], [1, NFULL * w]])
                )
            if i == 0:
                # group-shared runt resources, issued early enough to be
                # loaded before the runt matmuls ~30 matmuls later
                gr_t = gr_pool.tile([128, KW * GR], F16, tag="gr", name="gr_t")
                nc.sync.dma_start(gr_t[:], gr_dram[grp])
                xr_t = xr_pool.tile([128, w], F16, tag="xr", name="xr_t")
                for j in range(4):
                    cj = grp * 4 + j
                    nc.sync.dma_start(
                        xr_t[32 * j : 32 * j + KR, :],
                        _ap(xw_dram[cj], [[NWIN * w, KR], [1, w]],
                            extra_off=NFULL * w),
                    )
            for t in range(NFULL):
                ps_t = ps_pool.tile([128, w], F32, tag="ps", name="ps_t")
                # kw=PAD (shift 0) first: full-width start=True sets
                # has_written for the bank; shifted kws accumulate subranges.
                for idx, kw in enumerate(kws):
                    s = kw - PAD
                    w_lo = max(0, -s)
                    w_hi = w + min(0, -s)
                    lhs = g_t[:128, kw * GW + PAD : kw * GW + PAD + MT]
                    rhs = x_t[:128, t * w + w_lo + s : t * w + w_hi + s]
                    nc.tensor.matmul(
                        ps_t[:MT, w_lo:w_hi], lhs, rhs,
                        start=(idx == 0), stop=(idx == KW - 1),
                    )
                o_t = o_ts[oi % len(o_ts)]
                oi += 1
                nc.vector.tensor_copy(o_t[:MT, :], ps_t[:MT, :])
                # full-tile src + plain tensor-slice dst: HWDGE spreads this
                # across all 16 SDMA engines (a partial 122-row AP collapses
                # onto 2 engines). Rows 122..127 are junk the host ignores.
                nc.scalar.dma_start(y_dram[c, t], o_t[:, :])
        # one plain matmul per tap covers all 4 runts (block-diagonal G_runt)
        ps_r = ps_pool.tile([128, w], F32, tag="ps", name="ps_r")
        for idx, kw in enumerate(kws):
            s = kw - PAD
            w_lo = max(0, -s)
            w_hi = w + min(0, -s)
            lhs = gr_t[:128, kw * GR : kw * GR + GR]
            rhs = xr_t[:128, w_lo + s : w_hi + s]
            nc.tensor.matmul(
                ps_r[:GR, w_lo:w_hi], lhs, rhs,
                start=(idx == 0), stop=(idx == KW - 1),
            )
        o_r = or_pool.tile([128, w], F16, tag="or", name="o_r")
        nc.vector.tensor_copy(o_r[:GR, :], ps_r[:GR, :])
        nc.scalar.dma_start(yr_dram[grp], o_r[:GR, :])


def build_nc(cpc=CPC):
    w = W_DIM
    nc = bacc.Bacc(None, target_bir_lowering=False)

    xw_dram = nc.dram_tensor("XW", [cpc, 128, NWIN * w], F16, kind="ExternalInput")
    g_dram = nc.dram_tensor("G", [cpc, 128, KW * GW], F16, kind="ExternalInput")
    gr_dram = nc.dram_tensor("GRT", [NGRP, 128, KW * GR], F16, kind="ExternalInput")
    y_dram = nc.dram_tensor("Y", [cpc, NFULL, 128, w], F16, kind="ExternalOutput")
    yr_dram = nc.dram_tensor("YR", [NGRP, GR, w], F16, kind="ExternalOutput")

    with tile.TileContext(nc) as tc:
        with (
            tc.tile_pool(name="xw", bufs=1) as x_pool,
            tc.tile_pool(name="g", bufs=6) as g_pool,
            tc.tile_pool(name="gr", bufs=3) as gr_pool,
            tc.tile_pool(name="xr", bufs=3) as xr_pool,
            tc.tile_pool(name="ob", bufs=1) as o_pool,
            tc.tile_pool(name="orp", bufs=3) as or_pool,
            tc.tile_pool(name="ps", bufs=7, space="PSUM") as ps_pool,
        ):
            x_ts = [
                x_pool.tile([128, NFULL * w], F16, tag=f"x{i}", name=f"x{i}")
                for i in range(N_XBUF)
            ]
            o_ts = [
                o_pool.tile([128, w], F16, tag=f"o{i}", name=f"o{i}")
                for i in range(N_OBUF)
            ]
            # runt tiles keep partitions 27..31 etc. unwritten; zero them so
            # zero-weight lanes multiply 0, not stale NaN bits
            xr_zs = [
                xr_pool.tile([128, w], F16, tag="xr", name=f"xrz{i}")
                for i in range(3)
            ]
            for t in xr_zs:
                nc.vector.memset(t[:, :], 0.0)
            pools = (g_pool, gr_pool, xr_pool, or_pool, ps_pool)
            emit_body(nc, pools, x_ts, o_ts,
                      xw_dram, g_dram, gr_dram, y_dram, yr_dram)

    nc.compile()
    return nc


def build_g(wf):
    """wf: (C, 7, 7) filters -> (C, 128, 7, GW) fp16 banded matrices.

    G[c, j, kw, m2] = wf[c, j - m2 + 3, kw] where valid (0..6), else 0.
    Full tiles slice at g_off=PAD: lhsT[j, m] = wf[j - m] over padded input.
    """
    c = wf.shape[0]
    g = np.zeros((c, 128, KW, GW), dtype=NP_IN)
    js = np.arange(128)
    for kh in range(KH):
        m2 = js + 3 - kh
        mask = (m2 >= 0) & (m2 < GW)
        g[:, js[mask], :, m2[mask]] = wf[None, :, kh, :].astype(NP_IN)
    return g


def build_gr(wf):
    """wf: (C, 7, 7) -> (NGRP_total, 128, 7, GR) block-diagonal runt bands.

    Block i of group grp: rows 32i+jp (jp<27), cols 24i+mp (mp<24), value
    wf[4*grp+i, jp-mp, kw] when 0 <= jp-mp <= 6.
    """
    c = wf.shape[0]
    ngrp = c // 4
    gr = np.zeros((ngrp, 128, KW, GR), dtype=NP_IN)
    for i in range(4):
        for kh in range(KH):
            mp = np.arange(MR)
            jp = mp + kh
            sel = jp < KR
            # advanced indices separated by a slice: result axes are
            # (pair, group, kw), so broadcast the (group, kw) value into it
            gr[:, 32 * i + jp[sel], :, 24 * i + mp[sel]] = (
                wf[i::4, kh, :].astype(NP_IN)
            )
    return gr


def build_xw(x):
    """(C, H, W) f32 -> (C, 128, NWIN, W) fp16 window-major layout.

    Window t<4: padded rows 122t..122t+127 (pad = 3 zero rows on top).
    Window 4: padded rows 488..514 in slots 0..26, zeros elsewhere.
    """
    c, h, w = x.shape
    xp = np.zeros((c, HP, w), dtype=NP_IN)
    xp[:, PAD : PAD + h] = x.astype(NP_IN)
    xw = np.zeros((c, 128, NWIN, w), dtype=NP_IN)
    for t in range(NFULL):
        xw[:, :, t, :] = xp[:, MT * t : MT * t + 128, :]
    xw[:, :KR, NFULL, :] = xp[:, NFULL * MT : NFULL * MT + KR, :]
    return xw


_NC_CACHE = {}


def _get_nc():
    if CPC not in _NC_CACHE:
        _NC_CACHE[CPC] = build_nc(CPC)
    return _NC_CACHE[CPC]


def run(X, W, **spmd_kwargs):
    X = np.asarray(X, dtype=np.float32)
    W = np.asarray(W, dtype=np.float32)
    wf = np.ascontiguousarray(W[:, 0])  # (C, 7, 7)
    g_all = build_g(wf).reshape(C, 128, KW * GW)
    gr_all = build_gr(wf).reshape(C // 4, 128, KW * GR)
    xw_all = build_xw(X).reshape(C, 128, NWIN * W_DIM)

    nc = _get_nc()
    in_maps = []
    for core in range(N_CORES):
        c0 = core * CPC
        g0 = core * NGRP
        in_maps.append(
            {
                "XW": np.ascontiguousarray(xw_all[c0 : c0 + CPC]),
                "G": np.ascontiguousarray(g_all[c0 : c0 + CPC]),
                "GRT": np.ascontiguousarray(gr_all[g0 : g0 + NGRP]),
            }
        )
    res = run_bass_kernel_spmd(nc, in_maps, core_ids=list(range(N_CORES)),
                               **spmd_kwargs)
    y = np.empty((C, H, W_DIM), dtype=np.float32)
    for core in range(N_CORES):
        c0 = core * CPC
        yw = res.results[core]["Y"].reshape(CPC, NFULL, 128, W_DIM)
        yr = res.results[core]["YR"].reshape(NGRP, 4, MR, W_DIM)
        for t in range(NFULL):
            y[c0 : c0 + CPC, MT * t : MT * t + MT] = yw[:, t, :MT]
        y[c0 : c0 + CPC, NFULL * MT :] = yr.reshape(CPC, MR, W_DIM)
    return y, res


def kernel(X, W):
    return run(X, W)[0]


# revision 12
# speedup vs baseline: 1.8066x; 1.0001x over previous
"""Depthwise 7x7 conv (stride 1, pad 3) on 8 NeuronCores via Bass.

Strategy: channel-sharded SPMD (48 channels/core).  Per channel, conv along H
is a banded matmul on TensorE (stationary = banded filter matrix G, moving =
X rows); the 7 kw taps accumulate in PSUM via free-dim-shifted rhs slices.

v2 changes vs v1:
- The per-channel 24-row runt is computed for 4 channels at once in a SINGLE
  plain matmul per tap (block-diagonal G_runt, K=128/M=96) instead of 4
  tile_position-packed matmuls that share PE streaming bandwidth.
  1680 -> 1428 matmuls per core.
- Outputs stored as fp16 (half the write traffic); host converts to f32.
- X is pre-arranged on host into window-major layout [c, 128, 5, 512] so the
  big per-channel load is 4 KB contiguous per partition; Y uses a matching
  [c, 128, 4*512] layout (un-permuted on host).
- Runt windows live in a dedicated zeroed tile, shortening x-buffer lifetimes.
"""

import numpy as np

import concourse.bacc as bacc
import concourse.mybir as mybir
import concourse.tile as tile
from concourse.ap import AP
from concourse.bass_utils import run_bass_kernel_spmd

C, H, W_DIM = 384, 512, 512
KH = KW = 7
PAD = 3
N_CORES = 8
CPC = C // N_CORES  # 48 channels per core
NGRP = CPC // 4     # 12 runt groups per core

GW = 125   # master banded-matrix width (full blocks)
GR = 96    # runt band width: 4 channels x 24 output rows
HP = 520   # padded rows per channel (3 zero top + 512 + 5 zero tail)
MT = 122   # output rows per full tile
NFULL = 4  # full tiles per channel
MR = H - NFULL * MT  # runt output rows per channel (24)
KR = MR + PAD        # runt contraction rows (27)
NWIN = NFULL + 1     # windows in the host X layout (4 full + 1 runt slot)

import os as _os
N_XBUF = int(_os.environ.get("N_XBUF", "8"))
N_OBUF = int(_os.environ.get("N_OBUF", "10"))

F32 = mybir.dt.float32
F16 = mybir.dt.float16
NP_IN = np.float16


def _ap(base, dims, extra_off=0):
    return AP(tensor=base.tensor, offset=base.offset + extra_off, ap=list(dims))


def emit_body(nc, pools, x_ts, o_ts, warm_t,
              xw_dram, g_dram, gr_dram, y_dram, yr_dram):
    g_pool, gr_pool, xr_pool, or_pool, ps_pool = pools
    w = W_DIM
    kws = [PAD] + [k for k in range(KW) if k != PAD]
    ti = 0
    oi = 0
    # HAM warmup: ~60 tiny matmuls on a zeroed tile keep the PE busy during
    # the initial DMA head so the clock gate reaches 8/8 before the first
    # real matmul (otherwise ~16 matmuls run at 1.2 GHz).
    ps_w = ps_pool.tile([128, w], F32, tag="ps", name="ps_warm")
    for _ in range(60):
        nc.tensor.matmul(ps_w[:64, 0:64], warm_t[:128, 0:64],
                         warm_t[:128, 0:64], start=True, stop=True)
    for grp in range(NGRP):
        xr_t = None
        gr_t = None
        for i in range(4):
            c = grp * 4 + i
            g_t = g_pool.tile([128, KW * GW], F16, tag="g", name="g_t")
            x_t = x_ts[ti % len(x_ts)]
            ti += 1
            if grp == 0 and i == 0:
                # split the very first channel's loads so the first matmul
                # (tap kw=PAD on window 0) waits on ~160KB, not ~736KB
                k3 = PAD * GW
                nc.sync.dma_start(g_t[:, k3 : k3 + GW],
                                  _ap(g_dram[c], [[KW * GW, 128], [1, GW]],
                                      extra_off=k3))
                nc.sync.dma_start(
                    x_t[:, 0:w], _ap(xw_dram[c], [[NWIN * w, 128], [1, w]])
                )
                nc.sync.dma_start(g_t[:, 0:k3],
                                  _ap(g_dram[c], [[KW * GW, 128], [1, k3]]))
                nc.sync.dma_start(
                    g_t[:, k3 + GW :],
                    _ap(g_dram[c], [[KW * GW, 128], [1, (KW - PAD - 1) * GW]],
                        extra_off=k3 + GW),
                )
                nc.sync.dma_start(
                    x_t[:, w : NFULL * w],
                    _ap(xw_dram[c], [[NWIN * w, 128], [1, (NFULL - 1) * w]],
                        extra_off=w),
                )
            else:
                nc.sync.dma_start(g_t[:], g_dram[c])
                # 4 overlapping 128-row windows, 4KB contiguous per partition
                nc.sync.dma_start(
                    x_t[:], _ap(xw_dram[c], [[NWIN * w, 128], [1, NFULL * w]])
                )
            if i == 0:
                # group-shared runt resources, issued early enough to be
                # loaded before the runt matmuls ~30 matmuls later
                gr_t = gr_pool.tile([128, KW * GR], F16, tag="gr", name="gr_t")
                nc.sync.dma_start(gr_t[:], gr_dram[grp])
                xr_t = xr_pool.tile([128, w], F16, tag="xr", name="xr_t")
                for j in range(4):
                    cj = grp * 4 + j
                    nc.sync.dma_start(
                        xr_t[32 * j : 32 * j + KR, :],
                        _ap(xw_dram[cj], [[NWIN * w, KR], [1, w]],
                            extra_off=NFULL * w),
                    )
            for t in range(NFULL):
                ps_t = ps_pool.tile([128, w], F32, tag="ps", name="ps_t")
                # kw=PAD (shift 0) first: full-width start=True sets
                # has_written for the bank; shifted kws accumulate subranges.
                for idx, kw in enumerate(kws):
                    s = kw - PAD
                    w_lo = max(0, -s)
                    w_hi = w + min(0, -s)
                    lhs = g_t[:128, kw * GW + PAD : kw * GW + PAD + MT]
                    rhs = x_t[:128, t * w + w_lo + s : t * w + w_hi + s]
                    nc.tensor.matmul(
                        ps_t[:MT, w_lo:w_hi], lhs, rhs,
                        start=(idx == 0), stop=(idx == KW - 1),
                    )
                o_t = o_ts[oi % len(o_ts)]
                oi += 1
                nc.vector.tensor_copy(o_t[:MT, :], ps_t[:MT, :])
                # full-tile src + plain tensor-slice dst: HWDGE spreads this
                # across all 16 SDMA engines (a partial 122-row AP collapses
                # onto 2 engines). Rows 122..127 are junk the host ignores.
                nc.scalar.dma_start(y_dram[c, t], o_t[:, :])
            if i == 2:
                # one plain matmul per tap covers all 4 runts (block-diagonal
                # G_runt); emitted before the last channel so the kernel tail
                # is not gated on the runt chain
                ps_r = ps_pool.tile([128, w], F32, tag="ps", name="ps_r")
                for idx, kw in enumerate(kws):
                    s = kw - PAD
                    w_lo = max(0, -s)
                    w_hi = w + min(0, -s)
                    lhs = gr_t[:128, kw * GR : kw * GR + GR]
                    rhs = xr_t[:128, w_lo + s : w_hi + s]
                    nc.tensor.matmul(
                        ps_r[:GR, w_lo:w_hi], lhs, rhs,
                        start=(idx == 0), stop=(idx == KW - 1),
                    )
                o_r = or_pool.tile([128, w], F16, tag="or", name="o_r")
                nc.vector.tensor_copy(o_r[:GR, :], ps_r[:GR, :])
                nc.scalar.dma_start(yr_dram[grp], o_r[:GR, :])


def build_nc(cpc=CPC):
    w = W_DIM
    nc = bacc.Bacc(None, target_bir_lowering=False)

    xw_dram = nc.dram_tensor("XW", [cpc, 128, NWIN * w], F16, kind="ExternalInput")
    g_dram = nc.dram_tensor("G", [cpc, 128, KW * GW], F16, kind="ExternalInput")
    gr_dram = nc.dram_tensor("GRT", [NGRP, 128, KW * GR], F16, kind="ExternalInput")
    y_dram = nc.dram_tensor("Y", [cpc, NFULL, 128, w], F16, kind="ExternalOutput")
    yr_dram = nc.dram_tensor("YR", [NGRP, GR, w], F16, kind="ExternalOutput")

    with tile.TileContext(nc) as tc:
        with (
            tc.tile_pool(name="xw", bufs=1) as x_pool,
            tc.tile_pool(name="g", bufs=6) as g_pool,
            tc.tile_pool(name="gr", bufs=3) as gr_pool,
            tc.tile_pool(name="xr", bufs=3) as xr_pool,
            tc.tile_pool(name="ob", bufs=1) as o_pool,
            tc.tile_pool(name="orp", bufs=3) as or_pool,
            tc.tile_pool(name="ps", bufs=7, space="PSUM") as ps_pool,
        ):
            x_ts = [
                x_pool.tile([128, NFULL * w], F16, tag=f"x{i}", name=f"x{i}")
                for i in range(N_XBUF)
            ]
            o_ts = [
                o_pool.tile([128, w], F16, tag=f"o{i}", name=f"o{i}")
                for i in range(N_OBUF)
            ]
            # runt tiles keep partitions 27..31 etc. unwritten; zero them so
            # zero-weight lanes multiply 0, not stale NaN bits
            xr_zs = [
                xr_pool.tile([128, w], F16, tag="xr", name=f"xrz{i}")
                for i in range(3)
            ]
            for t in xr_zs:
                nc.vector.memset(t[:, :], 0.0)
            pools = (g_pool, gr_pool, xr_pool, or_pool, ps_pool)
            emit_body(nc, pools, x_ts, o_ts, xr_zs[0],
                      xw_dram, g_dram, gr_dram, y_dram, yr_dram)

    nc.compile()
    return nc


def build_g(wf):
    """wf: (C, 7, 7) filters -> (C, 128, 7, GW) fp16 banded matrices.

    G[c, j, kw, m2] = wf[c, j - m2 + 3, kw] where valid (0..6), else 0.
    Full tiles slice at g_off=PAD: lhsT[j, m] = wf[j - m] over padded input.
    """
    c = wf.shape[0]
    g = np.zeros((c, 128, KW, GW), dtype=NP_IN)
    js = np.arange(128)
    for kh in range(KH):
        m2 = js + 3 - kh
        mask = (m2 >= 0) & (m2 < GW)
        g[:, js[mask], :, m2[mask]] = wf[None, :, kh, :].astype(NP_IN)
    return g


def build_gr(wf):
    """wf: (C, 7, 7) -> (NGRP_total, 128, 7, GR) block-diagonal runt bands.

    Block i of group grp: rows 32i+jp (jp<27), cols 24i+mp (mp<24), value
    wf[4*grp+i, jp-mp, kw] when 0 <= jp-mp <= 6.
    """
    c = wf.shape[0]
    ngrp = c // 4
    gr = np.zeros((ngrp, 128, KW, GR), dtype=NP_IN)
    for i in range(4):
        for kh in range(KH):
            mp = np.arange(MR)
            jp = mp + kh
            sel = jp < KR
            # advanced indices separated by a slice: result axes are
            # (pair, group, kw), so broadcast the (group, kw) value into it
            gr[:, 32 * i + jp[sel], :, 24 * i + mp[sel]] = (
                wf[i::4, kh, :].astype(NP_IN)
            )
    return gr


def build_xw(x):
    """(C, H, W) f32 -> (C, 128, NWIN, W) fp16 window-major layout.

    Window t<4: padded rows 122t..122t+127 (pad = 3 zero rows on top).
    Window 4: padded rows 488..514 in slots 0..26, zeros elsewhere.
    """
    c, h, w = x.shape
    xp = np.zeros((c, HP, w), dtype=NP_IN)
    xp[:, PAD : PAD + h] = x.astype(NP_IN)
    xw = np.zeros((c, 128, NWIN, w), dtype=NP_IN)
    for t in range(NFULL):
        xw[:, :, t, :] = xp[:, MT * t : MT * t + 128, :]
    xw[:, :KR, NFULL, :] = xp[:, NFULL * MT : NFULL * MT + KR, :]
    return xw


_NC_CACHE = {}


def _get_nc():
    if CPC not in _NC_CACHE:
        _NC_CACHE[CPC] = build_nc(CPC)
    return _NC_CACHE[CPC]


def run(X, W, **spmd_kwargs):
    X = np.asarray(X, dtype=np.float32)
    W = np.asarray(W, dtype=np.float32)
    wf = np.ascontiguousarray(W[:, 0])  # (C, 7, 7)
    g_all = build_g(wf).reshape(C, 128, KW * GW)
    gr_all = build_gr(wf).reshape(C // 4, 128, KW * GR)
    xw_all = build_xw(X).reshape(C, 128, NWIN * W_DIM)

    nc = _get_nc()
    in_maps = []
    for core in range(N_CORES):
        c0 = core * CPC
        g0 = core * NGRP
        in_maps.append(
            {
                "XW": np.ascontiguousarray(xw_all[c0 : c0 + CPC]),
                "G": np.ascontiguousarray(g_all[c0 : c0 + CPC]),
                "GRT": np.ascontiguousarray(gr_all[g0 : g0 + NGRP]),
            }
        )
    res = run_bass_kernel_spmd(nc, in_maps, core_ids=list(range(N_CORES)),
                               **spmd_kwargs)
    y = np.empty((C, H, W_DIM), dtype=np.float32)
    for core in range(N_CORES):
        c0 = core * CPC
        yw = res.results[core]["Y"].reshape(CPC, NFULL, 128, W_DIM)
        yr = res.results[core]["YR"].reshape(NGRP, 4, MR, W_DIM)
        for t in range(NFULL):
            y[c0 : c0 + CPC, MT * t : MT * t + MT] = yw[:, t, :MT]
        y[c0 : c0 + CPC, NFULL * MT :] = yr.reshape(CPC, MR, W_DIM)
    return y, res


def kernel(X, W):
    return run(X, W)[0]


# revision 14
# speedup vs baseline: 1.8085x; 1.0010x over previous
"""Depthwise 7x7 conv (stride 1, pad 3) on 8 NeuronCores via Bass.

Strategy: channel-sharded SPMD (48 channels/core).  Per channel, conv along H
is a banded matmul on TensorE (stationary = banded filter matrix G, moving =
X rows); the 7 kw taps accumulate in PSUM via free-dim-shifted rhs slices.

v2 changes vs v1:
- The per-channel 24-row runt is computed for 4 channels at once in a SINGLE
  plain matmul per tap (block-diagonal G_runt, K=128/M=96) instead of 4
  tile_position-packed matmuls that share PE streaming bandwidth.
  1680 -> 1428 matmuls per core.
- Outputs stored as fp16 (half the write traffic); host converts to f32.
- X is pre-arranged on host into window-major layout [c, 128, 5, 512] so the
  big per-channel load is 4 KB contiguous per partition; Y uses a matching
  [c, 128, 4*512] layout (un-permuted on host).
- Runt windows live in a dedicated zeroed tile, shortening x-buffer lifetimes.
"""

import numpy as np

import concourse.bacc as bacc
import concourse.mybir as mybir
import concourse.tile as tile
from concourse.ap import AP
from concourse.bass_utils import run_bass_kernel_spmd

C, H, W_DIM = 384, 512, 512
KH = KW = 7
PAD = 3
N_CORES = 8
CPC = C // N_CORES  # 48 channels per core
NGRP = CPC // 4     # 12 runt groups per core

GW = 125   # master banded-matrix width (full blocks)
GR = 96    # runt band width: 4 channels x 24 output rows
HP = 520   # padded rows per channel (3 zero top + 512 + 5 zero tail)
MT = 122   # output rows per full tile
NFULL = 4  # full tiles per channel
MR = H - NFULL * MT  # runt output rows per channel (24)
KR = MR + PAD        # runt contraction rows (27)
NWIN = NFULL + 1     # windows in the host X layout (4 full + 1 runt slot)

import os as _os
N_XBUF = int(_os.environ.get("N_XBUF", "8"))
N_OBUF = int(_os.environ.get("N_OBUF", "10"))

F32 = mybir.dt.float32
F16 = mybir.dt.float16
NP_IN = np.float16


def _ap(base, dims, extra_off=0):
    return AP(tensor=base.tensor, offset=base.offset + extra_off, ap=list(dims))


def emit_body(nc, pools, x_ts, o_ts, warm_t,
              xw_dram, g_dram, gr_dram, y_dram, yr_dram):
    g_pool, gr_pool, xr_pool, or_pool, ps_pool = pools
    w = W_DIM
    kws = [PAD] + [k for k in range(KW) if k != PAD]
    ti = 0
    oi = 0
    # HAM warmup: dependency-free matmuls on an uninitialized scratch tile
    # keep the PE busy from the end of its preamble (~4us) through the DMA
    # head (~11us) so the clock gate reaches 8/8 before the first real
    # matmul. The scratch PSUM result is never read and each real matmul
    # group re-opens its bank with start=True, so garbage values are inert.
    ps_w = ps_pool.tile([128, w], F32, tag="ps", name="ps_warm")
    for _ in range(56):
        nc.tensor.matmul(ps_w[:64, 0:128], warm_t[:128, 0:64],
                         warm_t[:128, 0:128], start=True, stop=True)
    for grp in range(NGRP):
        xr_t = None
        gr_t = None
        for i in range(4):
            c = grp * 4 + i
            g_t = g_pool.tile([128, KW * GW], F16, tag="g", name="g_t")
            x_t = x_ts[ti % len(x_ts)]
            ti += 1
            if grp == 0 and i == 0:
                # split the very first channel's loads so the first matmul
                # (tap kw=PAD on window 0) waits on ~160KB, not ~736KB
                k3 = PAD * GW
                nc.sync.dma_start(g_t[:, k3 : k3 + GW],
                                  _ap(g_dram[c], [[KW * GW, 128], [1, GW]],
                                      extra_off=k3))
                nc.sync.dma_start(
                    x_t[:, 0:w], _ap(xw_dram[c], [[NWIN * w, 128], [1, w]])
                )
                nc.sync.dma_start(g_t[:, 0:k3],
                                  _ap(g_dram[c], [[KW * GW, 128], [1, k3]]))
                nc.sync.dma_start(
                    g_t[:, k3 + GW :],
                    _ap(g_dram[c], [[KW * GW, 128], [1, (KW - PAD - 1) * GW]],
                        extra_off=k3 + GW),
                )
                nc.sync.dma_start(
                    x_t[:, w : NFULL * w],
                    _ap(xw_dram[c], [[NWIN * w, 128], [1, (NFULL - 1) * w]],
                        extra_off=w),
                )
            else:
                nc.sync.dma_start(g_t[:], g_dram[c])
                # 4 overlapping 128-row windows, 4KB contiguous per partition
                nc.sync.dma_start(
                    x_t[:], _ap(xw_dram[c], [[NWIN * w, 128], [1, NFULL * w]])
                )
            if i == 0:
                # group-shared runt resources, issued early enough to be
                # loaded before the runt matmuls ~30 matmuls later
                gr_t = gr_pool.tile([128, KW * GR], F16, tag="gr", name="gr_t")
                nc.sync.dma_start(gr_t[:], gr_dram[grp])
                xr_t = xr_pool.tile([128, w], F16, tag="xr", name="xr_t")
                for j in range(4):
                    cj = grp * 4 + j
                    nc.sync.dma_start(
                        xr_t[32 * j : 32 * j + KR, :],
                        _ap(xw_dram[cj], [[NWIN * w, KR], [1, w]],
                            extra_off=NFULL * w),
                    )
            for t in range(NFULL):
                ps_t = ps_pool.tile([128, w], F32, tag="ps", name="ps_t")
                # kw=PAD (shift 0) first: full-width start=True sets
                # has_written for the bank; shifted kws accumulate subranges.
                for idx, kw in enumerate(kws):
                    s = kw - PAD
                    w_lo = max(0, -s)
                    w_hi = w + min(0, -s)
                    lhs = g_t[:128, kw * GW + PAD : kw * GW + PAD + MT]
                    rhs = x_t[:128, t * w + w_lo + s : t * w + w_hi + s]
                    nc.tensor.matmul(
                        ps_t[:MT, w_lo:w_hi], lhs, rhs,
                        start=(idx == 0), stop=(idx == KW - 1),
                    )
                o_t = o_ts[oi % len(o_ts)]
                oi += 1
                nc.vector.tensor_copy(o_t[:MT, :], ps_t[:MT, :])
                # full-tile src + plain tensor-slice dst: HWDGE spreads this
                # across all 16 SDMA engines (a partial 122-row AP collapses
                # onto 2 engines). Rows 122..127 are junk the host ignores.
                nc.scalar.dma_start(y_dram[c, t], o_t[:, :])
            if i == 2:
                # one plain matmul per tap covers all 4 runts (block-diagonal
                # G_runt); emitted before the last channel so the kernel tail
                # is not gated on the runt chain
                ps_r = ps_pool.tile([128, w], F32, tag="ps", name="ps_r")
                for idx, kw in enumerate(kws):
                    s = kw - PAD
                    w_lo = max(0, -s)
                    w_hi = w + min(0, -s)
                    lhs = gr_t[:128, kw * GR : kw * GR + GR]
                    rhs = xr_t[:128, w_lo + s : w_hi + s]
                    nc.tensor.matmul(
                        ps_r[:GR, w_lo:w_hi], lhs, rhs,
                        start=(idx == 0), stop=(idx == KW - 1),
                    )
                o_r = or_pool.tile([128, w], F16, tag="or", name="o_r")
                nc.vector.tensor_copy(o_r[:GR, :], ps_r[:GR, :])
                nc.scalar.dma_start(yr_dram[grp], o_r[:GR, :])


def build_nc(cpc=CPC):
    w = W_DIM
    nc = bacc.Bacc(None, target_bir_lowering=False)

    xw_dram = nc.dram_tensor("XW", [cpc, 128, NWIN * w], F16, kind="ExternalInput")
    g_dram = nc.dram_tensor("G", [cpc, 128, KW * GW], F16, kind="ExternalInput")
    gr_dram = nc.dram_tensor("GRT", [NGRP, 128, KW * GR], F16, kind="ExternalInput")
    y_dram = nc.dram_tensor("Y", [cpc, NFULL, 128, w], F16, kind="ExternalOutput")
    yr_dram = nc.dram_tensor("YR", [NGRP, GR, w], F16, kind="ExternalOutput")

    with tile.TileContext(nc) as tc:
        with (
            tc.tile_pool(name="xw", bufs=1) as x_pool,
            tc.tile_pool(name="g", bufs=6) as g_pool,
            tc.tile_pool(name="gr", bufs=3) as gr_pool,
            tc.tile_pool(name="xr", bufs=3) as xr_pool,
            tc.tile_pool(name="ob", bufs=1) as o_pool,
            tc.tile_pool(name="orp", bufs=3) as or_pool,
            tc.tile_pool(name="warm", bufs=1) as w_pool,
            tc.tile_pool(name="ps", bufs=7, space="PSUM") as ps_pool,
        ):
            x_ts = [
                x_pool.tile([128, NFULL * w], F16, tag=f"x{i}", name=f"x{i}")
                for i in range(N_XBUF)
            ]
            o_ts = [
                o_pool.tile([128, w], F16, tag=f"o{i}", name=f"o{i}")
                for i in range(N_OBUF)
            ]
            # runt tiles keep partitions 27..31 etc. unwritten; zero them so
            # zero-weight lanes multiply 0, not stale NaN bits
            xr_zs = [
                xr_pool.tile([128, w], F16, tag="xr", name=f"xrz{i}")
                for i in range(3)
            ]
            for t in xr_zs:
                nc.vector.memset(t[:, :], 0.0)
            warm_t = w_pool.tile([128, 128], F16, tag="warm", name="warm")
            nc.gpsimd.memset(warm_t[:, :], 0.0)
            pools = (g_pool, gr_pool, xr_pool, or_pool, ps_pool)
            emit_body(nc, pools, x_ts, o_ts, warm_t,
                      xw_dram, g_dram, gr_dram, y_dram, yr_dram)

    nc.compile()
    return nc


def build_g(wf):
    """wf: (C, 7, 7) filters -> (C, 128, 7, GW) fp16 banded matrices.

    G[c, j, kw, m2] = wf[c, j - m2 + 3, kw] where valid (0..6), else 0.
    Full tiles slice at g_off=PAD: lhsT[j, m] = wf[j - m] over padded input.
    """
    c = wf.shape[0]
    g = np.zeros((c, 128, KW, GW), dtype=NP_IN)
    js = np.arange(128)
    for kh in range(KH):
        m2 = js + 3 - kh
        mask = (m2 >= 0) & (m2 < GW)
        g[:, js[mask], :, m2[mask]] = wf[None, :, kh, :].astype(NP_IN)
    return g


def build_gr(wf):
    """wf: (C, 7, 7) -> (NGRP_total, 128, 7, GR) block-diagonal runt bands.

    Block i of group grp: rows 32i+jp (jp<27), cols 24i+mp (mp<24), value
    wf[4*grp+i, jp-mp, kw] when 0 <= jp-mp <= 6.
    """
    c = wf.shape[0]
    ngrp = c // 4
    gr = np.zeros((ngrp, 128, KW, GR), dtype=NP_IN)
    for i in range(4):
        for kh in range(KH):
            mp = np.arange(MR)
            jp = mp + kh
            sel = jp < KR
            # advanced indices separated by a slice: result axes are
            # (pair, group, kw), so broadcast the (group, kw) value into it
            gr[:, 32 * i + jp[sel], :, 24 * i + mp[sel]] = (
                wf[i::4, kh, :].astype(NP_IN)
            )
    return gr


def build_xw(x):
    """(C, H, W) f32 -> (C, 128, NWIN, W) fp16 window-major layout.

    Window t<4: padded rows 122t..122t+127 (pad = 3 zero rows on top).
    Window 4: padded rows 488..514 in slots 0..26, zeros elsewhere.
    """
    c, h, w = x.shape
    xp = np.zeros((c, HP, w), dtype=NP_IN)
    xp[:, PAD : PAD + h] = x.astype(NP_IN)
    xw = np.zeros((c, 128, NWIN, w), dtype=NP_IN)
    for t in range(NFULL):
        xw[:, :, t, :] = xp[:, MT * t : MT * t + 128, :]
    xw[:, :KR, NFULL, :] = xp[:, NFULL * MT : NFULL * MT + KR, :]
    return xw


_NC_CACHE = {}


def _get_nc():
    if CPC not in _NC_CACHE:
        _NC_CACHE[CPC] = build_nc(CPC)
    return _NC_CACHE[CPC]


def run(X, W, **spmd_kwargs):
    X = np.asarray(X, dtype=np.float32)
    W = np.asarray(W, dtype=np.float32)
    wf = np.ascontiguousarray(W[:, 0])  # (C, 7, 7)
    g_all = build_g(wf).reshape(C, 128, KW * GW)
    gr_all = build_gr(wf).reshape(C // 4, 128, KW * GR)
    xw_all = build_xw(X).reshape(C, 128, NWIN * W_DIM)

    nc = _get_nc()
    in_maps = []
    for core in range(N_CORES):
        c0 = core * CPC
        g0 = core * NGRP
        in_maps.append(
            {
                "XW": np.ascontiguousarray(xw_all[c0 : c0 + CPC]),
                "G": np.ascontiguousarray(g_all[c0 : c0 + CPC]),
                "GRT": np.ascontiguousarray(gr_all[g0 : g0 + NGRP]),
            }
        )
    res = run_bass_kernel_spmd(nc, in_maps, core_ids=list(range(N_CORES)),
                               **spmd_kwargs)
    y = np.empty((C, H, W_DIM), dtype=np.float32)
    for core in range(N_CORES):
        c0 = core * CPC
        yw = res.results[core]["Y"].reshape(CPC, NFULL, 128, W_DIM)
        yr = res.results[core]["YR"].reshape(NGRP, 4, MR, W_DIM)
        for t in range(NFULL):
            y[c0 : c0 + CPC, MT * t : MT * t + MT] = yw[:, t, :MT]
        y[c0 : c0 + CPC, NFULL * MT :] = yr.reshape(CPC, MR, W_DIM)
    return y, res


def kernel(X, W):
    return run(X, W)[0]
